# revision 1
# baseline (speedup 1.0000x reference)
"""Causal self-attention Trainium2 kernel (8-core SPMD).

Problem: x[4,2048,1024] @ w_qkv[1024,3072] -> per-head causal attention
(16 heads, hd=64) -> ctx @ w_out[1024,1024].

Sharding (8 cores): core c handles batch b = c//2 and head-group
g = c%2 (8 heads). Each core computes a partial output
x[b] @ ... @ w_out[rows of its heads]; host sums the two partials per
batch (tensor-parallel row-split of w_out).

Device algorithm (per core), all matmuls bf16 with fp32 PSUM accumulate:
  waves (per 512-column slice jj of the sequence):
    xT = x^T             (PE transpose via identity, bf16)
    v  = x @ w_v         (packed into per-head ones-augmented lhsT "vaug")
    qkT[:, :, jj] = (x @ w_qk)^T   (computed transposed: w_qk^T x^T)
  attention (query-block j outer, head pair t inner; pairs row-packed
  in 64-row strips of the PE array):
    scoresT[sk,sq] = k_h^T q_h     (row-packed K=64 matmul pairs)
    expT = exp(scale*scoresT)      (ACT; causal diag masked via bf16 mul)
    ctxT_aug[128,sq] = [v_h | 1]^T @ expT  (rows 0:64 ctx, 64:128 sums)
    ctxT = ctxT_aug[0:64] * recip(ctxT_aug[64:128])
    out rows of block j = ctxT^T @ w_out_rows   (partial; host reduces)
"""

import threading

import numpy as np

S = 2048
D = 1024
B = 4
NCORES = 8
ST = 128           # seq tile (partitions)
NS = S // ST       # 16
SQ = 512           # query-block width (matmul free dim)
NJ = S // SQ       # 4
ND = D // 128      # 8 contraction tiles
NPAIR = 4          # head pairs per core
SCALE = 0.125      # 1/sqrt(64)

_cache = {}
_lock = threading.Lock()


def build_nc(reps=1):
    from contextlib import ExitStack, nullcontext

    import concourse.mybir as mybir
    import concourse.tile as tile
    from concourse import bacc
    from concourse.masks import make_identity

    f32 = mybir.dt.float32
    bf16 = mybir.dt.bfloat16

    nc = bacc.Bacc("TRN2", target_bir_lowering=False, debug=False)

    x = nc.dram_tensor("x", [S, D], f32, kind="ExternalInput").ap()
    wqk = nc.dram_tensor("wqk", [D, 1024], f32, kind="ExternalInput").ap()
    wv = nc.dram_tensor("wv", [D, 512], f32, kind="ExternalInput").ap()
    wout = nc.dram_tensor("wout", [512, D], f32, kind="ExternalInput").ap()
    out = nc.dram_tensor("out", [S, D], f32, kind="ExternalOutput").ap()

    with ExitStack() as ctx:
        tc = ctx.enter_context(tile.TileContext(nc))
        const = ctx.enter_context(tc.tile_pool(name="const", bufs=1))
        persist = ctx.enter_context(tc.tile_pool(name="persist", bufs=1))
        expp = ctx.enter_context(tc.tile_pool(name="expp", bufs=6))
        recp = ctx.enter_context(tc.tile_pool(name="recp", bufs=2))

        # --- constants ---
        ident = const.tile([128, 128], bf16)
        make_identity(nc, ident)
        # Diagonal causal masks for (sq=512)-wide exp tiles holding two
        # 128-row sk blocks: mask[p, w, c] = 1 if c - p - 128*(d0+w) >= 0.
        m01 = const.tile([128, 2, SQ], bf16)
        m23 = const.tile([128, 2, SQ], bf16)
        for m, base in ((m01, 0), (m23, -256)):
            nc.vector.memset(m, 1.0)
            nc.gpsimd.affine_select(
                out=m, in_=m, compare_op=mybir.AluOpType.is_ge, fill=0.0,
                base=base, channel_multiplier=-1, pattern=[[-128, 2], [1, SQ]],
            )

        # --- persistent tensors ---
        xT = persist.tile([128, ND, S], bf16)            # x^T, d on partitions
        qkT = persist.tile([128, 8, S], bf16)            # tiles 0-3 q pairs, 4-7 k
        vaug = persist.tile([128, 8, NS, 128], bf16)     # per head: [v | ones]
        ctxT = persist.tile([128, NPAIR, S], bf16)       # normalized ctx^T
        wqk_bf = persist.tile([128, ND, 1024], bf16)
        wv_bf = persist.tile([128, ND, 512], bf16)
        wout_bf = persist.tile([128, NPAIR, D], bf16)

        nc.vector.memset(vaug, 1.0)  # ones columns; v halves overwritten below
        # prime the ACT exp table set at t=0 so the ~2.7us table load
        # hides under the projection waves instead of delaying attention
        warm = const.tile([128, 1], f32)
        nc.scalar.activation(warm, ident[:, 0:1],
                             mybir.ActivationFunctionType.Exp, scale=1.0)

        # repeat body for steady-state timing (reps>1: timing builds only)
        with (tc.For_i(0, reps, 1) if reps > 1 else nullcontext()):
            with (
                tc.tile_pool(name="stage", bufs=3) as stage,
                # single shared 1-bank psum tag for waves + out-projection
                tc.tile_pool(name="wps", bufs=2, space="PSUM") as wps,
                # attention psums: scores (2 banks x 2) + ctx (1 bank x 2)
                tc.tile_pool(name="atps", bufs=2, space="PSUM") as atps,
            ):
                # v weights first (v projection consumes them early)
                for ki in range(ND):
                    r = slice(128 * ki, 128 * ki + 128)
                    wv_st = stage.tile([128, 512], f32, tag="wv_st", bufs=2)
                    nc.sync.dma_start(wv_st, wv[r, :])
                    nc.vector.tensor_copy(wv_bf[:, ki, :], wv_st)

                def wave_xv(jj):
                    for si in range(4 * jj, 4 * jj + 4):
                        r = slice(128 * si, 128 * si + 128)
                        x_st = stage.tile([128, 1024], f32, tag="x_st")
                        nc.sync.dma_start(x_st, x[r, :])
                        x_bf = stage.tile([128, 1024], bf16, tag="x_bf")
                        nc.vector.tensor_copy(x_bf, x_st)
                        for grp in range(2):  # d-tile groups of 4
                            ps = wps.tile([128, 4, 128], bf16, tag="ps")
                            for dq in range(4):
                                di = 4 * grp + dq
                                nc.tensor.transpose(
                                    ps[:, dq, :],
                                    x_bf[:, 128 * di:128 * di + 128], ident,
                                )
                            nc.vector.tensor_copy(
                                xT[:, 4 * grp:4 * grp + 4, r], ps
                            )
                        # v projection for this s-tile
                        psv = wps.tile([128, 512], f32, tag="ps")
                        for ki in range(ND):
                            nc.tensor.matmul(
                                psv, xT[:, ki, r], wv_bf[:, ki, :],
                                start=(ki == 0), stop=(ki == ND - 1),
                            )
                        nc.vector.tensor_copy(
                            vaug[:, 0:8, si, 0:64],
                            psv.rearrange("p (h e) -> p h e", h=8),
                        )
                def wave_qk(jj):
                    c = slice(SQ * jj, SQ * jj + SQ)
                    # qkT columns of this wave; order (q_t, k_t) pairs so
                    # attention pair t unblocks after 2 tiles, not 8
                    for tq in (0, 4, 1, 5, 2, 6, 3, 7):
                        ps = wps.tile([128, SQ], f32, tag="ps")
                        for ki in range(ND):
                            nc.tensor.matmul(
                                ps, wqk_bf[:, ki, 128 * tq:128 * tq + 128],
                                xT[:, ki, c],
                                start=(ki == 0), stop=(ki == ND - 1),
                            )
                        nc.vector.tensor_copy(qkT[:, tq, c], ps)

                wave_xv(0)
                # qk weights: after wave 0's x tiles, before wave 0's qkT
                for ki in range(ND):
                    r = slice(128 * ki, 128 * ki + 128)
                    wqk_st = stage.tile([128, 1024], f32, tag="wqk_st", bufs=2)
                    nc.sync.dma_start(wqk_st, wqk[r, :])
                    nc.vector.tensor_copy(wqk_bf[:, ki, :], wqk_st)
                wave_qk(0)
                for t in range(NPAIR):
                    r = slice(128 * t, 128 * t + 128)
                    wo_st = stage.tile([128, 1024], f32, tag="wqk_st", bufs=2)
                    nc.sync.dma_start(wo_st, wout[r, :])
                    nc.vector.tensor_copy(wout_bf[:, t, :], wo_st)
                for jj in range(1, NJ):
                    wave_xv(jj)
                    wave_qk(jj)

                # --- attention: j outer, pairs inner; out-proj per j ---
                for j in range(NJ):
                    c = slice(SQ * j, SQ * j + SQ)
                    nblk = 4 * j + 4
                    for t in range(NPAIR):
                        hA, hB = 2 * t, 2 * t + 1
                        psCA = atps.tile([128, SQ], f32, tag="ctx")
                        psCB = atps.tile([128, SQ], f32, tag="ctx")
                        for ip in range(nblk // 2):
                            # m23 pair (sk offsets 256/384 into the query
                            # block): valid region is columns 256:512 only;
                            # compute at N=256 and mask with m01's pattern.
                            hi = (2 * ip == 4 * j + 2)
                            co = 256 if hi else 0       # column offset
                            cw = SQ - co                # width
                            cq = slice(SQ * j + co, SQ * j + SQ)
                            psSA = atps.tile([128, 2, SQ], f32, tag="score")
                            psSB = atps.tile([128, 2, SQ], f32, tag="score")
                            for w in range(2):
                                i = 2 * ip + w
                                ks = slice(128 * i, 128 * i + 128)
                                nc.tensor.matmul(
                                    psSA[:, w, 0:cw], qkT[0:64, 4 + t, ks],
                                    qkT[0:64, t, cq], start=True, stop=True,
                                    tile_position=(0, 0),
                                )
                                nc.tensor.matmul(
                                    psSB[:, w, 0:cw], qkT[64:128, 4 + t, ks],
                                    qkT[64:128, t, cq], start=True, stop=True,
                                    tile_position=(64, 0),
                                )
                            expA = expp.tile([128, 2, SQ], bf16, tag="exp")
                            expB = expp.tile([128, 2, SQ], bf16, tag="exp")
                            nc.scalar.activation(
                                expA[:, :, 0:cw], psSA[:, :, 0:cw],
                                mybir.ActivationFunctionType.Exp,
                                scale=SCALE,
                            )
                            nc.scalar.activation(
                                expB[:, :, 0:cw], psSB[:, :, 0:cw],
                                mybir.ActivationFunctionType.Exp,
                                scale=SCALE,
                            )
                            if 2 * ip >= 4 * j:  # diagonal pair: causal mask
                                m = m01[:, :, 0:cw] if hi else m01
                                nc.vector.tensor_mul(
                                    expA[:, :, 0:cw], expA[:, :, 0:cw], m
                                )
                                nc.vector.tensor_mul(
                                    expB[:, :, 0:cw], expB[:, :, 0:cw], m
                                )
                            for w in range(2):
                                i = 2 * ip + w
                                nc.tensor.matmul(
                                    psCA[:, co:SQ], vaug[:, hA, i, :],
                                    expA[:, w, 0:cw],
                                    start=(i == 0), stop=(i == nblk - 1),
                                )
                                nc.tensor.matmul(
                                    psCB[:, co:SQ], vaug[:, hB, i, :],
                                    expB[:, w, 0:cw],
                                    start=(i == 0), stop=(i == nblk - 1),
                                )
                        # normalize: ctx rows 0:64 / sums rows 64:128
                        recA = recp.tile([128, SQ], f32, tag="rec")
                        recB = recp.tile([128, SQ], f32, tag="rec")
                        nc.vector.reciprocal(recA[64:128, :], psCA[64:128, :])
                        nc.vector.tensor_mul(
                            ctxT[0:64, t, c], psCA[0:64, :], recA[64:128, :]
                        )
                        nc.vector.reciprocal(recB[64:128, :], psCB[64:128, :])
                        nc.vector.tensor_mul(
                            ctxT[64:128, t, c], psCB[0:64, :], recB[64:128, :]
                        )
                    # output projection for the 4 s-tiles of this j
                    for si in range(4 * j, 4 * j + 4):
                        r = slice(128 * si, 128 * si + 128)
                        o_st = stage.tile([128, 1024], f32, tag="o_st", bufs=2)
                        for n in range(2):
                            pso = wps.tile([128, 512], f32, tag="ps")
                            for t in range(NPAIR):
                                nc.tensor.matmul(
                                    pso, ctxT[:, t, r],
                                    wout_bf[:, t, 512 * n:512 * n + 512],
                                    start=(t == 0), stop=(t == NPAIR - 1),
                                )
                            nc.vector.tensor_copy(
                                o_st[:, 512 * n:512 * n + 512], pso
                            )
                        nc.sync.dma_start(out[r, :], o_st)

    if not nc.is_finalized():
        nc.finalize()
    return nc


def make_in_maps(x, w_qkv, w_out):
    x = np.ascontiguousarray(np.asarray(x, dtype=np.float32))
    w_qkv = np.ascontiguousarray(np.asarray(w_qkv, dtype=np.float32))
    w_out = np.ascontiguousarray(np.asarray(w_out, dtype=np.float32))
    in_maps = []
    for c in range(NCORES):
        b, g = c // 2, c % 2
        cs = slice(512 * g, 512 * g + 512)
        in_maps.append({
            "x": x[b],
            "wqk": np.ascontiguousarray(
                np.concatenate([w_qkv[:, 512 * g:512 * g + 512],
                                w_qkv[:, 1024 + 512 * g:1024 + 512 * g + 512]],
                               axis=1)),
            "wv": np.ascontiguousarray(w_qkv[:, 2048 + 512 * g:2048 + 512 * g + 512]),
            "wout": np.ascontiguousarray(w_out[cs, :]),
        })
    return in_maps


def run_sharded(inputs, trace=False, trace_kwargs=None):
    """Run on 8 neuron cores; returns (out[B,S,D], BassKernelResults)."""
    from concourse import bass_utils

    with _lock:
        if "nc" not in _cache:
            _cache["nc"] = build_nc()
    nc = _cache["nc"]
    in_maps = make_in_maps(**inputs)
    res = bass_utils.run_bass_kernel_spmd(
        nc, in_maps, core_ids=list(range(NCORES)),
        trace=trace, **(trace_kwargs or {}),
    )
    outs = np.stack(
        [res.results[2 * b]["out"] + res.results[2 * b + 1]["out"]
         for b in range(B)]
    ).astype(np.float32)
    return outs, res


def kernel(x, w_qkv, w_out):
    out, _ = run_sharded({"x": x, "w_qkv": w_qkv, "w_out": w_out})
    return out



# revision 11
# speedup vs baseline: 1.1516x; 1.1516x over previous
"""Causal self-attention Trainium2 kernel (8-core SPMD).

Problem: x[4,2048,1024] @ w_qkv[1024,3072] -> per-head causal attention
(16 heads, hd=64) -> ctx @ w_out[1024,1024].

Sharding (8 cores): core c handles batch b = c//2 and head-group
g = c%2 (8 heads). Each core computes a partial output
x[b] @ ... @ w_out[rows of its heads]; host sums the two partials per
batch (tensor-parallel row-split of w_out).

Device algorithm (per core), all matmuls bf16 with fp32 PSUM accumulate:
  waves (per 512-column slice jj of the sequence):
    xT = x^T             (PE transpose via identity, bf16)
    v  = x @ w_v         (packed into per-head ones-augmented lhsT "vaug")
    qkT[:, :, jj] = (x @ w_qk)^T   (computed transposed: w_qk^T x^T)
  attention (query-block j outer, head pair t inner; pairs row-packed
  in 64-row strips of the PE array):
    scoresT[sk,sq] = k_h^T q_h     (row-packed K=64 matmul pairs)
    expT = exp(scale*scoresT)      (ACT; causal diag zeroed via Pool
                                    affine_select, in place on the tile)
    ctxT_aug[128,sq] = [v_h | 1]^T @ expT  (rows 0:64 ctx, 64:128 sums)
    ctxT = ctxT_aug[0:64] * recip(ctxT_aug[64:128])
    out rows of block j = ctxT^T @ w_out_rows   (partial; host reduces)

Scheduling: attention block j is software-pipelined (scores one step
ahead of ctx) and interleaved with projection wave j+1 (and with the
deferred output projections during the last block), so the ACT-engine
exp stream hides under PE matmul work instead of serializing after it.
Dtype conversions and causal masking run on the otherwise-idle Pool
engine.
"""

import threading

import numpy as np

S = 2048
D = 1024
B = 4
NCORES = 8
ST = 128           # seq tile (partitions)
NS = S // ST       # 16
SQ = 512           # query-block width (matmul free dim)
NJ = S // SQ       # 4
ND = D // 128      # 8 contraction tiles
NPAIR = 4          # head pairs per core
SCALE = 0.125      # 1/sqrt(64)

_cache = {}
_lock = threading.Lock()


def build_nc(reps=1):
    from contextlib import ExitStack, nullcontext

    import concourse.mybir as mybir
    import concourse.tile as tile
    from concourse import bacc
    from concourse.masks import make_identity

    f32 = mybir.dt.float32
    bf16 = mybir.dt.bfloat16

    nc = bacc.Bacc("TRN2", target_bir_lowering=False, debug=False)

    x = nc.dram_tensor("x", [S, D], f32, kind="ExternalInput").ap()
    wqk = nc.dram_tensor("wqk", [D, 1024], f32, kind="ExternalInput").ap()
    wv = nc.dram_tensor("wv", [D, 512], f32, kind="ExternalInput").ap()
    wout = nc.dram_tensor("wout", [512, D], f32, kind="ExternalInput").ap()
    out = nc.dram_tensor("out", [S, D], f32, kind="ExternalOutput").ap()

    with ExitStack() as ctx:
        tc = ctx.enter_context(tile.TileContext(nc))
        const = ctx.enter_context(tc.tile_pool(name="const", bufs=1))
        persist = ctx.enter_context(tc.tile_pool(name="persist", bufs=1))
        expp = ctx.enter_context(tc.tile_pool(name="expp", bufs=6))
        recp = ctx.enter_context(tc.tile_pool(name="recp", bufs=2))

        ident = const.tile([128, 128], bf16)
        make_identity(nc, ident)
        # Causal mask for the two diagonal k-blocks of a step: the invalid
        # region of an exp tile [p, w, c] (c - p - 128w < 0) always lies in
        # columns 0:256, so a [128, 2, 256] multiplicand suffices.
        m01 = const.tile([128, 2, 256], bf16)
        nc.vector.memset(m01, 1.0)
        nc.gpsimd.affine_select(
            out=m01, in_=m01, compare_op=mybir.AluOpType.is_ge, fill=0.0,
            base=0, channel_multiplier=-1, pattern=[[-128, 2], [1, 256]],
        )

        # --- persistent tensors ---
        xT = persist.tile([128, ND, S], bf16)            # x^T, d on partitions
        qkT = persist.tile([128, 8, S], bf16)            # tiles 0-3 q pairs, 4-7 k
        vaug = persist.tile([128, 8, NS, 128], bf16)     # per head: [v | ones]
        ctxT = persist.tile([128, NPAIR, S], bf16)       # normalized ctx^T
        wqk_bf = persist.tile([128, ND, 1024], bf16)
        wv_bf = persist.tile([128, ND, 512], bf16)
        wout_bf = persist.tile([128, NPAIR, D], bf16)

        # ones columns only (v halves are written by the waves; keeping the
        # memset off them avoids a write-after-write wait on first use)
        nc.vector.memset(vaug[:, 0:8, 0:NS, 64:128], 1.0)
        # prime the ACT exp table set so the ~2.7us table load hides under
        # the projection waves instead of delaying the first attention exp
        warm = const.tile([128, 1], f32)
        nc.scalar.activation(warm, ident[:, 0:1],
                             mybir.ActivationFunctionType.Exp, scale=1.0)

        # repeat body for steady-state timing (reps>1: timing builds only)
        with (tc.For_i(0, reps, 1) if reps > 1 else nullcontext()):
            with (
                tc.tile_pool(name="stage", bufs=3) as stage,
                # single shared 1-bank psum tag for waves + out-projection
                tc.tile_pool(name="wps", bufs=2, space="PSUM") as wps,
                # attention psums: scores (2 banks x 2) + ctx (1 bank x 2)
                tc.tile_pool(name="atps", bufs=2, space="PSUM") as atps,
            ):
                def load_weights(dst, src, ntiles, width, tag, eng=None):
                    for ki in range(ntiles):
                        r = slice(128 * ki, 128 * ki + 128)
                        st = stage.tile([128, width], f32, tag=tag, bufs=2)
                        nc.sync.dma_start(st, src[r, :])
                        (eng or nc.gpsimd).tensor_copy(dst[:, ki, :], st)

                # --- wave units (PE filler closures) ---
                def unit_xv(si):
                    def emit():
                        r = slice(128 * si, 128 * si + 128)
                        x_st = stage.tile([128, 1024], f32, tag="x_st")
                        nc.sync.dma_start(x_st, x[r, :])
                        x_bf = stage.tile([128, 1024], bf16, tag="x_bf")
                        nc.vector.tensor_copy(x_bf, x_st)
                        for grp in range(2):  # d-tile groups of 4
                            ps = wps.tile([128, 4, 128], bf16, tag="ps")
                            for dq in range(4):
                                di = 4 * grp + dq
                                nc.tensor.transpose(
                                    ps[:, dq, :],
                                    x_bf[:, 128 * di:128 * di + 128], ident,
                                )
                            nc.vector.tensor_copy(
                                xT[:, 4 * grp:4 * grp + 4, r], ps
                            )
                        # v projection for this s-tile
                        psv = wps.tile([128, 512], f32, tag="ps")
                        for ki in range(ND):
                            nc.tensor.matmul(
                                psv, xT[:, ki, r], wv_bf[:, ki, :],
                                start=(ki == 0), stop=(ki == ND - 1),
                            )
                        nc.vector.tensor_copy(
                            vaug[:, 0:8, si, 0:64],
                            psv.rearrange("p (h e) -> p h e", h=8),
                        )
                    return emit

                def unit_qk(jj, tq):
                    def emit():
                        c = slice(SQ * jj, SQ * jj + SQ)
                        ps = wps.tile([128, SQ], f32, tag="ps")
                        for ki in range(ND):
                            nc.tensor.matmul(
                                ps, wqk_bf[:, ki, 128 * tq:128 * tq + 128],
                                xT[:, ki, c],
                                start=(ki == 0), stop=(ki == ND - 1),
                            )
                        nc.vector.tensor_copy(qkT[:, tq, c], ps)
                    return emit

                def unit_out(si):
                    def emit():
                        r = slice(128 * si, 128 * si + 128)
                        o_st = stage.tile([128, 1024], f32, tag="o_st", bufs=2)
                        for n in range(2):
                            pso = wps.tile([128, 512], f32, tag="ps")
                            for t in range(NPAIR):
                                nc.tensor.matmul(
                                    pso, ctxT[:, t, r],
                                    wout_bf[:, t, 512 * n:512 * n + 512],
                                    start=(t == 0), stop=(t == NPAIR - 1),
                                )
                            nc.vector.tensor_copy(
                                o_st[:, 512 * n:512 * n + 512], pso
                            )
                        nc.sync.dma_start(out[r, :], o_st)
                    return emit

                def wave_units(jj):
                    us = [unit_xv(si) for si in range(4 * jj, 4 * jj + 4)]
                    # (q_t, k_t) pair order so attention pair t unblocks
                    # after 2 tiles, not 8
                    us += [unit_qk(jj, tq) for tq in (0, 4, 1, 5, 2, 6, 3, 7)]
                    return us

                # --- attention steps, software-pipelined one deep ---
                def attn_block(j, fillers):
                    c = slice(SQ * j, SQ * j + SQ)
                    nblk = 4 * j + 4
                    steps = [(t, ip) for t in range(NPAIR)
                             for ip in range(nblk // 2)]
                    nsteps = len(steps)
                    # spread fillers across steps
                    fidx = [0] * (nsteps + 1)
                    for s in range(nsteps + 1):
                        fidx[s] = (len(fillers) * (s + 1)) // (nsteps + 1)
                    state = {}   # t -> (psCA, psCB)
                    pend = None  # (t, ip, expA, expB, hi, co, cw)

                    def emit_scores(t, ip):
                        hi = (2 * ip == 4 * j + 2)
                        co = 256 if hi else 0
                        cw = SQ - co
                        cq = slice(SQ * j + co, SQ * j + SQ)
                        psSA = atps.tile([128, 2, SQ], f32, tag="score")
                        psSB = atps.tile([128, 2, SQ], f32, tag="score")
                        for w in range(2):
                            i = 2 * ip + w
                            ks = slice(128 * i, 128 * i + 128)
                            nc.tensor.matmul(
                                psSA[:, w, 0:cw], qkT[0:64, 4 + t, ks],
                                qkT[0:64, t, cq], start=True, stop=True,
                                tile_position=(0, 0),
                            )
                            nc.tensor.matmul(
                                psSB[:, w, 0:cw], qkT[64:128, 4 + t, ks],
                                qkT[64:128, t, cq], start=True, stop=True,
                                tile_position=(64, 0),
                            )
                        expA = expp.tile([128, 2, SQ], bf16, tag="exp")
                        expB = expp.tile([128, 2, SQ], bf16, tag="exp")
                        diag = 2 * ip >= 4 * j
                        for e, psS in ((expA, psSA), (expB, psSB)):
                            nc.scalar.activation(
                                e[:, :, 0:cw], psS[:, :, 0:cw],
                                mybir.ActivationFunctionType.Exp,
                                scale=SCALE,
                            )
                            if diag:
                                nc.vector.tensor_mul(
                                    e[:, :, 0:256], e[:, :, 0:256], m01,
                                )
                        return (t, ip, expA, expB, hi, co, cw)

                    def emit_ctx(p):
                        t, ip, expA, expB, hi, co, cw = p
                        hA, hB = 2 * t, 2 * t + 1
                        psCA, psCB = state[t]
                        for w in range(2):
                            i = 2 * ip + w
                            nc.tensor.matmul(
                                psCA[:, co:SQ], vaug[:, hA, i, :],
                                expA[:, w, 0:cw],
                                start=(i == 0), stop=(i == nblk - 1),
                            )
                            nc.tensor.matmul(
                                psCB[:, co:SQ], vaug[:, hB, i, :],
                                expB[:, w, 0:cw],
                                start=(i == 0), stop=(i == nblk - 1),
                            )
                        if 2 * ip + 1 == nblk - 1:  # last k-block of pair t
                            recA = recp.tile([128, SQ], f32, tag="rec")
                            recB = recp.tile([128, SQ], f32, tag="rec")
                            nc.vector.reciprocal(
                                recA[64:128, :], psCA[64:128, :])
                            nc.vector.tensor_mul(
                                ctxT[0:64, t, c], psCA[0:64, :],
                                recA[64:128, :]
                            )
                            nc.vector.reciprocal(
                                recB[64:128, :], psCB[64:128, :])
                            nc.vector.tensor_mul(
                                ctxT[64:128, t, c], psCB[0:64, :],
                                recB[64:128, :]
                            )

                    for s, (t, ip) in enumerate(steps):
                        if ip == 0:
                            state[t] = (
                                atps.tile([128, SQ], f32, tag="ctx",
                                          name="psCA"),
                                atps.tile([128, SQ], f32, tag="ctx",
                                          name="psCB"),
                            )
                        nxt = emit_scores(t, ip)
                        for f in fillers[fidx[s - 1] if s else 0:fidx[s]]:
                            f()
                        if pend is not None:
                            emit_ctx(pend)
                        pend = nxt
                    for f in fillers[fidx[nsteps - 1]:fidx[nsteps]]:
                        f()
                    if pend is not None:
                        emit_ctx(pend)

                # --- emission schedule ---
                load_weights(wv_bf, wv, ND, 512, "wv_st")
                for u in wave_units(0)[:4]:      # wave_xv(0)
                    u()
                load_weights(wqk_bf, wqk, ND, 1024, "wqk_st", eng=nc.vector)
                for u in wave_units(0)[4:]:      # wave_qk(0)
                    u()
                load_weights(wout_bf, wout, NPAIR, 1024, "wo_st")

                attn_block(0, wave_units(1))
                attn_block(1, wave_units(2))
                attn_block(2, wave_units(3))
                # last block: filler = deferred output projections of j=0..2
                attn_block(3, [unit_out(si) for si in range(12)])
                for si in range(12, 16):
                    unit_out(si)()

    if not nc.is_finalized():
        nc.finalize()
    return nc


def make_in_maps(x, w_qkv, w_out):
    x = np.ascontiguousarray(np.asarray(x, dtype=np.float32))
    w_qkv = np.ascontiguousarray(np.asarray(w_qkv, dtype=np.float32))
    w_out = np.ascontiguousarray(np.asarray(w_out, dtype=np.float32))
    in_maps = []
    for c in range(NCORES):
        b, g = c // 2, c % 2
        cs = slice(512 * g, 512 * g + 512)
        in_maps.append({
            "x": x[b],
            "wqk": np.ascontiguousarray(
                np.concatenate([w_qkv[:, 512 * g:512 * g + 512],
                                w_qkv[:, 1024 + 512 * g:1024 + 512 * g + 512]],
                               axis=1)),
            "wv": np.ascontiguousarray(w_qkv[:, 2048 + 512 * g:2048 + 512 * g + 512]),
            "wout": np.ascontiguousarray(w_out[cs, :]),
        })
    return in_maps


def run_sharded(inputs, trace=False, trace_kwargs=None):
    """Run on 8 neuron cores; returns (out[B,S,D], BassKernelResults)."""
    from concourse import bass_utils

    with _lock:
        if "nc" not in _cache:
            _cache["nc"] = build_nc()
    nc = _cache["nc"]
    in_maps = make_in_maps(**inputs)
    res = bass_utils.run_bass_kernel_spmd(
        nc, in_maps, core_ids=list(range(NCORES)),
        trace=trace, **(trace_kwargs or {}),
    )
    outs = np.stack(
        [res.results[2 * b]["out"] + res.results[2 * b + 1]["out"]
         for b in range(B)]
    ).astype(np.float32)
    return outs, res


def kernel(x, w_qkv, w_out):
    out, _ = run_sharded({"x": x, "w_qkv": w_qkv, "w_out": w_out})
    return out
